# revision 16
# baseline (speedup 1.0000x reference)
"""AppearanceDecoder Trainium2 kernel — 8-core data-parallel over batch.

v6: transposed-score formulation, fp16 feats/weights.
Per batch b, per level l with feat F [Cin, D] (fp16 in DRAM), conv w [256, Cin]:
    reference: fp = w @ F + bias; S = outq @ fp; A = softmax_d(S); q_l = A @ fp^T
    Conv bias drops out of the scores (softmax row-const) and becomes additive
    in q_l. Everything is computed in d-on-partition layout:
        psf[d, 0:256]   = fpT = F^T @ w^T        (contract Cin)
        psf[d, 256:356] = ST  = F^T @ (w^T outq^T)
    ONE fused matmul per 128-d chunk produces both (rhs = [wT | owT] packed
    per level), so no e-transposes are needed at all:
        eT = exp(ST - SHIFT)                  (ACT, d-on-partition)
        qp[q, 0:257] += eT^T @ [fpT | 1]      (ones column gives Z = sum_d e)
        q_l = qp[:, :256] / qp[:, 256] + bias_l
    ow = outq @ [w0|w1|w2] is computed once up front ([Q, 1792], free dim
    448-wide at 1 cyc/row) and PE-transposed into each level's rhs pack.
Then aq = concat(q_0..q_2); 2-layer agg MLP; LayerNorm; 3-layer proj MLP, all
in fp16 operands (1 cyc/row at any free size) with f32 PSUM accumulation.

DMA: feats are fp16 in DRAM (host-converted) -> 14.7MB/core, ~41us at
358GB/s, far under the PE's ~90us. All of L1/L2 (+ all weight packs) are
prefetched during L0's compute, so only L0 streams against the clock.
Copies of fpT out of PSUM alternate Vector/GpSimd; exp on ACT.
"""
import numpy as np
from contextlib import ExitStack

import concourse.bass as bass
import concourse.tile as tile
from concourse import bacc, mybir
from concourse.masks import make_identity

F32 = mybir.dt.float32
F32R = mybir.dt.float32r
F16 = mybir.dt.float16
AF = mybir.ActivationFunctionType

Q = 100
B = 8
C = 256
LEVELS = [(256, 16384), (512, 4096), (1024, 1024)]  # (Cin, D)
CIN_ALL = 1792
SHIFT = 88.0
N_CORES = 8

FEAT_BUFS = [7, 4, 1]
# paramsE (f16, [128, 3584]): aggw1T[1536] aggw2T[512] projw1T[512] projw2T[512] projw3T[512]
PE_OFF = [0, 1536, 2048, 2560, 3072, 3584]
# paramsf (f32, [128, 16]): bcat[6] aggb1[2] aggb2[2] projb1[2] projb2[2] projb3[2]
PF_OFF = [0, 6, 8, 10, 12, 14, 16]
# paramsW (f16, [128, 3584]): wT packs per level, [128, kc, 256] each
PW_OFF = [0, 512, 1536, 3584]


def _mm(nc, out, lhsT, rhs, start, stop):
    nc.tensor.matmul(out, lhsT, rhs, start=start, stop=stop)


def build_graph():
    nc = bacc.Bacc("TRN2", target_bir_lowering=False, debug=False)

    feats = [
        nc.dram_tensor(f"feat{l}", [cin, d], F16, kind="ExternalInput").ap()
        for l, (cin, d) in enumerate(LEVELS)
    ]
    params0 = nc.dram_tensor("params0", [128, 3784], F16, kind="ExternalInput").ap()
    paramsW = nc.dram_tensor("paramsW", [128, 3584], F16, kind="ExternalInput").ap()
    paramsE = nc.dram_tensor("paramsE", [128, 3584], F16, kind="ExternalInput").ap()
    paramsf = nc.dram_tensor("paramsf", [128, 16], F32, kind="ExternalInput").ap()
    out_d = nc.dram_tensor("out", [C, Q], F32, kind="ExternalOutput").ap()

    with tile.TileContext(nc) as tc, ExitStack() as ctx:
        const = ctx.enter_context(tc.tile_pool(name="const", bufs=1))

        identF = const.tile([128, 128], F32)
        make_identity(nc, identF)
        identH = const.tile([128, 128], F16)
        nc.vector.tensor_copy(identH, identF)
        negc = const.tile([128, 1], F32)
        nc.vector.memset(negc, -SHIFT)
        ones_c = const.tile([128, 4], F32)
        nc.vector.memset(ones_c, 1.0)
        ones_v = ones_c.rearrange("p (a b) -> p a b", a=2)
        aq = const.tile([Q, 3 * C], F16)
        aqT = const.tile([128, 6, Q], F16)
        ow_sb = const.tile([Q, CIN_ALL], F16)

        p0_sb = const.tile([128, 3784], F16)
        pf_sb = const.tile([128, 16], F32)
        pE_sb = const.tile([128, 3584], F16)
        owTwT = [
            const.tile([128, cin // 128, 356], F16, name=f"owTwT{l}")
            for l, (cin, _) in enumerate(LEVELS)
        ]

        p0v = p0_sb.rearrange("p (a x) -> p a x", a=2)  # [:, oc, 0:100]=outqT
        aggw1T_sb = pE_sb[:, PE_OFF[0]:PE_OFF[1]].rearrange("p (k o) -> p k o", o=C)
        aggw2T_sb = pE_sb[:, PE_OFF[1]:PE_OFF[2]].rearrange("p (k o) -> p k o", o=C)
        projwT_sb = [
            pE_sb[:, PE_OFF[2 + i]:PE_OFF[3 + i]].rearrange("p (k o) -> p k o", o=C)
            for i in range(3)
        ]
        bcat_sb = pf_sb[:, PF_OFF[0]:PF_OFF[1]]
        aggb1_sb = pf_sb[:, PF_OFF[1]:PF_OFF[2]]
        aggb2_sb = pf_sb[:, PF_OFF[2]:PF_OFF[3]]
        projb_sb = [pf_sb[:, PF_OFF[3 + i]:PF_OFF[4 + i]] for i in range(3)]

        with ExitStack() as mctx:
            psP = mctx.enter_context(tc.tile_pool(name="psP", bufs=2, space="PSUM"))
            psQ = mctx.enter_context(tc.tile_pool(name="psQ", bufs=2, space="PSUM"))
            psT = mctx.enter_context(tc.tile_pool(name="psT", bufs=2, space="PSUM"))
            epool = mctx.enter_context(tc.tile_pool(name="e", bufs=4))
            fpool = mctx.enter_context(tc.tile_pool(name="fp", bufs=4))
            fpools = [
                mctx.enter_context(
                    tc.tile_pool(name=f"ft{lvl}", bufs=FEAT_BUFS[lvl])
                )
                for lvl in range(3)
            ]

            # ---- DMA front: first feat tile, then weight packs ----
            f_rs = [
                feats[lvl].rearrange("(j p) d -> p j d", p=128) for lvl in range(3)
            ]
            ft0_first = fpools[0].tile([128, 2, 1024], F16, name="ft0_0", tag="ft")
            for j in range(2):
                nc.sync.dma_start(out=ft0_first[:, j, :], in_=f_rs[0][:, j, 0:1024])
            nc.sync.dma_start(out=p0_sb, in_=params0)
            nc.sync.dma_start(
                out=owTwT[0][:, :, 0:256],
                in_=paramsW[:, PW_OFF[0]:PW_OFF[1]].rearrange(
                    "p (j c) -> p j c", c=256),
            )
            nc.sync.dma_start(out=pf_sb, in_=paramsf)

            ft1_pre = [
                fpools[1].tile([128, 4, 1024], F16, name=f"ft1_{i}", tag="ft")
                for i in range(4)
            ]
            ft2_pre = fpools[2].tile([128, 8, 1024], F16, name="ft2", tag="ft")

            # prefetch work items drained inside the L0 loop (1-2 per tile)
            pre = []
            for lvl in (1, 2):
                pre.append((owTwT[lvl][:, :, 0:256],
                            paramsW[:, PW_OFF[lvl]:PW_OFF[lvl + 1]].rearrange(
                                "p (j c) -> p j c", c=256)))
            for i in range(4):
                for j in range(4):
                    pre.append((ft1_pre[i][:, j, :],
                                f_rs[1][:, j, i * 1024:(i + 1) * 1024]))
            for j in range(8):
                pre.append((ft2_pre[:, j, :], f_rs[2][:, j, 0:1024]))
            for c4 in range(4):
                pre.append((pE_sb[:, c4 * 896:(c4 + 1) * 896],
                            paramsE[:, c4 * 896:(c4 + 1) * 896]))

            # ---- PE warm-up while the packs land (flips HAM gate to 8/8) ----
            for i in range(48):
                warm = psT.tile([128, Q], F16, name=f"warm{i}", tag="tp")
                nc.tensor.transpose(warm, identH[:Q, :128], identH[:Q, :Q])

            # ---- ow = outq @ [w0|w1|w2]  ([Q, 1792], then PE-transpose) ----
            for t in range(4):
                po = psT.tile([Q, 448], F32, name=f"po{t}", tag="tp")
                for oc in range(2):
                    _mm(nc, po, p0v[:, oc, 0:100],
                        p0v[:, oc, 100 + t * 448:100 + (t + 1) * 448],
                        oc == 0, oc == 1)
                nc.scalar.copy(out=ow_sb[:, t * 448:(t + 1) * 448], in_=po)
            cch = 0
            for lvl, (cin, _) in enumerate(LEVELS):
                for j in range(cin // 128):
                    tp = psT.tile([128, Q], F16, name=f"owt{cch}", tag="tp")
                    nc.tensor.transpose(
                        tp, ow_sb[:, cch * 128:(cch + 1) * 128], identH[:Q, :Q])
                    nc.scalar.copy(out=owTwT[lvl][:, j, 256:356], in_=tp)
                    cch += 1

            # ---- main: fused [fpT|ST] matmul -> exp -> qp accumulation ----
            for lvl, (cin, dd) in enumerate(LEVELS):
                kc = cin // 128
                nd2 = dd // 1024
                G = dd // 128
                qp = psQ.tile([Q, 258], F32, name=f"qp{lvl}", tag="qp")
                g = 0
                for n2 in range(nd2):
                    if lvl == 0:
                        if n2 == 0:
                            ft = ft0_first
                        else:
                            ft = fpools[0].tile(
                                [128, 2, 1024], F16, name=f"ft0_{n2}", tag="ft")
                            for j in range(2):
                                nc.sync.dma_start(
                                    out=ft[:, j, :],
                                    in_=f_rs[0][:, j, n2 * 1024:(n2 + 1) * 1024])
                            # drain 2 prefetch DMAs behind each L0 tile
                            for _ in range(2):
                                if pre:
                                    o_, i_ = pre.pop(0)
                                    nc.sync.dma_start(out=o_, in_=i_)
                    elif lvl == 1:
                        ft = ft1_pre[n2]
                    else:
                        ft = ft2_pre
                    for mp in range(4):
                        # chunk pair in one 2-bank PSUM tile: exp + fpT copy
                        # amortize their fixed access/decode cost over 2 chunks
                        psf2 = psP.tile([128, 2, 512], F32,
                                        name=f"s{lvl}_{g}", tag="f")
                        for h in range(2):
                            m = mp * 2 + h
                            for j in range(kc):
                                _mm(nc, psf2[:, h, 0:356],
                                    ft[:, j, m * 128:(m + 1) * 128],
                                    owTwT[lvl][:, j, :], j == 0, j == kc - 1)
                        eT2 = epool.tile([128, 2, Q], F32R,
                                         name=f"e{lvl}_{g}", tag="e")
                        nc.scalar.activation(
                            out=eT2, in_=psf2[:, :, 256:356], func=AF.Exp,
                            bias=negc, scale=1.0)
                        fpT2 = fpool.tile([128, 2, 258], F32R,
                                          name=f"fp{lvl}_{g}", tag="fp")
                        nc.vector.tensor_copy(fpT2[:, :, 0:256],
                                              psf2[:, :, 0:256])
                        nc.gpsimd.tensor_copy(fpT2[:, :, 256:258], ones_v)
                        for h in range(2):
                            _mm(nc, qp, eT2[:, h, :], fpT2[:, h, :],
                                g == 0, g == G - 1)
                            g += 1

                # level tail: normalize by Z, add conv bias, build aqT
                r_t = const.tile([Q, 1], F32, name=f"rt{lvl}")
                nc.vector.reciprocal(out=r_t, in_=qp[:, 256:257])
                nc.vector.tensor_scalar_mul(
                    aq[:, lvl * C:(lvl + 1) * C], qp[:, 0:256], r_t)
                for kk in range(2):
                    k = 2 * lvl + kk
                    tpq = psT.tile([128, Q], F16, name=f"tpq{k}", tag="tp")
                    nc.tensor.transpose(
                        tpq, aq[:, k * 128:(k + 1) * 128], identH[:Q, :Q])
                    nc.scalar.activation(
                        out=aqT[:, k, :], in_=tpq, func=AF.Identity,
                        bias=bcat_sb[:, k:k + 1], scale=1.0)

        # ---- epilogue: agg MLP -> LN -> proj MLP, channel-on-partition ----
        with ExitStack() as ectx:
            ep = ectx.enter_context(tc.tile_pool(name="ep", bufs=1))
            psE = ectx.enter_context(tc.tile_pool(name="psE", bufs=3, space="PSUM"))

            def dense_T(src, w_sb, b_sb, func, out_dtype, nk, name):
                dst = ep.tile([128, 2, Q], out_dtype, name=name)
                for oc in range(2):
                    pz = psE.tile([128, Q], F32, name=f"{name}_p{oc}", tag="eps")
                    for k in range(nk):
                        _mm(nc, pz, w_sb[:, k, oc * 128:(oc + 1) * 128],
                            src[:, k, :], k == 0, k == nk - 1)
                    nc.scalar.activation(
                        out=dst[:, oc, :], in_=pz, func=func,
                        bias=b_sb[:, oc:oc + 1], scale=1.0)
                return dst

            z1T = dense_T(aqT, aggw1T_sb, aggb1_sb, AF.Relu, F16, 6, "z1T")
            z2T = dense_T(z1T, aggw2T_sb, aggb2_sb, AF.Identity, F16, 2, "z2T")

            z2 = ep.tile([Q, C], F32)
            for k in range(2):
                tpz = psE.tile([Q, 128], F16, name=f"tpz{k}", tag="eps")
                nc.tensor.transpose(tpz, z2T[:, k, :], identH[:128, :128])
                nc.vector.tensor_copy(z2[:, k * 128:(k + 1) * 128], tpz)
            stats = ep.tile([Q, 6], F32)
            nc.vector.bn_stats(out=stats, in_=z2)
            mv = ep.tile([Q, 2], F32)
            nc.vector.bn_aggr(out=mv, in_=stats)
            eps_t = ep.tile([Q, 1], F32)
            nc.vector.memset(eps_t, 1e-5)
            sd = ep.tile([Q, 1], F32)
            nc.scalar.activation(out=sd, in_=mv[:, 1:2], func=AF.Sqrt,
                                 bias=eps_t, scale=1.0)
            rstd = ep.tile([Q, 1], F32)
            nc.vector.reciprocal(out=rstd, in_=sd)
            zn = ep.tile([Q, C], F16)
            nc.vector.tensor_scalar(
                out=zn, in0=z2, scalar1=mv[:, 0:1], scalar2=rstd,
                op0=mybir.AluOpType.subtract, op1=mybir.AluOpType.mult,
            )

            znT = ep.tile([128, 2, Q], F16)
            for k in range(2):
                tpn = psE.tile([128, Q], F16, name=f"tpn{k}", tag="eps")
                nc.tensor.transpose(
                    tpn, zn[:, k * 128:(k + 1) * 128], identH[:Q, :Q])
                nc.scalar.copy(out=znT[:, k, :], in_=tpn)

            zp1 = dense_T(znT, projwT_sb[0], projb_sb[0], AF.Relu, F16, 2, "zp1")
            zp2 = dense_T(zp1, projwT_sb[1], projb_sb[1], AF.Relu, F16, 2, "zp2")
            zp3 = dense_T(zp2, projwT_sb[2], projb_sb[2], AF.Identity, F32, 2, "zp3")
            nc.sync.dma_start(
                out=out_d.rearrange("(a p) q -> p a q", p=128), in_=zp3
            )

    nc.compile()
    return nc


_GRAPH = None


def _get_graph():
    global _GRAPH
    if _GRAPH is None:
        _GRAPH = build_graph()
    return _GRAPH


def _tile_p(a):
    """[r*128, X] -> [128, r, X] host pre-tiling (partition-major packing)."""
    r = a.shape[0] // 128
    return np.ascontiguousarray(a.reshape(r, 128, -1).transpose(1, 0, 2))


def _vec_p(v):
    """[r*128] -> [128, r]"""
    r = v.shape[0] // 128
    return v.reshape(r, 128).T


def make_in_maps(output, feat0, feat1, feat2,
                 w0, b0, w1, b1, w2, b2, ln_g, ln_b,
                 agg_w1, agg_b1, agg_w2, agg_b2,
                 proj_w1, proj_b1, proj_w2, proj_b2, proj_w3, proj_b3):
    f32, f16 = np.float32, np.float16
    w0, w1, w2 = (np.asarray(x, f32) for x in (w0, w1, w2))
    wcat_t = _tile_p(np.concatenate([w0, w1, w2], axis=1).astype(f16))
    pW = np.ascontiguousarray(np.concatenate(
        [_tile_p(np.asarray(w, f32).T.astype(f16)).reshape(128, -1)
         for w in (w0, w1, w2)], axis=1))
    lng_v = np.asarray(ln_g, f32)
    pw1g = np.asarray(proj_w1, f32) * lng_v[None, :]
    pE = np.ascontiguousarray(np.concatenate(
        [_tile_p(np.asarray(w, f32).T.astype(f16)).reshape(128, -1)
         for w in (agg_w1, agg_w2, pw1g, proj_w2, proj_w3)], axis=1))
    pf = np.ascontiguousarray(np.concatenate(
        [_vec_p(np.asarray(v, f32)) for v in
         (np.concatenate([b0, b1, b2]), agg_b1, agg_b2,
          np.asarray(proj_w1, f32) @ np.asarray(ln_b, f32) + proj_b1,
          proj_b2, proj_b3)], axis=1), dtype=f32)
    shared = {"paramsW": pW, "paramsE": pE, "paramsf": pf}
    feats = [feat0, feat1, feat2]
    in_maps = []
    for b in range(N_CORES):
        m = dict(shared)
        outqT = _tile_p(np.asarray(output, f32)[:, b, :].T.astype(f16))
        m["params0"] = np.ascontiguousarray(
            np.concatenate([outqT, wcat_t], axis=2).reshape(128, 3784))
        for l, (cin, d) in enumerate(LEVELS):
            m[f"feat{l}"] = np.ascontiguousarray(
                np.asarray(feats[l][b], f32).reshape(cin, d).astype(f16))
        in_maps.append(m)
    return in_maps


def kernel(output, feat0, feat1, feat2, output_mask,
           w0, b0, w1, b1, w2, b2, ln_g, ln_b,
           agg_w1, agg_b1, agg_w2, agg_b2,
           proj_w1, proj_b1, proj_w2, proj_b2, proj_w3, proj_b3,
           **_unused):
    from concourse.bass_utils import run_bass_kernel_spmd

    nc = _get_graph()
    in_maps = make_in_maps(
        output, feat0, feat1, feat2, w0, b0, w1, b1, w2, b2, ln_g, ln_b,
        agg_w1, agg_b1, agg_w2, agg_b2,
        proj_w1, proj_b1, proj_w2, proj_b2, proj_w3, proj_b3,
    )
    res = run_bass_kernel_spmd(nc, in_maps, core_ids=list(range(N_CORES)))
    return np.stack([res.results[b]["out"].T for b in range(N_CORES)], axis=1)
